# revision 39
# baseline (speedup 1.0000x reference)
"""Trainium2 Bass kernel for nn_MemoryQueueContrastiveLoss.

Strategy (8 NeuronCores):
  - Shard the QUEUE dimension (65536 -> 8 x 8192) across cores; replicate the
    batch features.  Each core computes partial queue negative sums
    (sum_q exp(s/t)) for ALL 1024 batch rows over its queue shard, plus the
    batch-vs-batch part for its own 128-row / 128-col shard.
  - Two ReduceScatter collectives combine the per-core partial sums so core k
    receives exactly its row-shard slice of the global negative sums.
  - Each core then computes its shard of the final loss terms
      log1p(neg * exp(-s)) = ln(exp(s) + neg) - s
    and returns per-partition partial sums; the host adds 8x[128] partials.

All transcendentals (exp/ln) run on the ACT engine, which is the bottleneck
(~2*B*Q/8 = 16.8M exps/core).  Matmuls run as float32r (full PE rate).
"""

import sys

for _p in ("/opt/trn_rl_repo",):
    if _p not in sys.path:
        sys.path.insert(0, _p)

import numpy as np

import concourse.bass as bass  # noqa: F401  (registers types)
import concourse.bacc as bacc
import concourse.mybir as mybir
from concourse import tile
from concourse import bass_utils

B = 1024          # batch
D = 128           # feature dim
Q = 65536         # queue size
NCORES = 8
QS = Q // NCORES  # 8192 queue columns per core
RT = B // 128     # 8 row tiles
INIT_TEMP = 0.07
MAX_TEMP = 0.07 * 1.3

F32 = mybir.dt.float32
F32R = mybir.dt.float32r
AF = mybir.ActivationFunctionType
ALU = mybir.AluOpType
AX = mybir.AxisListType

# ACT tile width for the queue exp grind: 2048 fp32 = 4 PSUM banks.
GW = 2048
NG = QS // GW     # 4 grind chunks per row tile
NMM = GW // 512   # 4 matmuls per grind chunk


def _f32r(ap):
    return ap.bitcast(F32R)


def build(eff_temp: float, queue_weight: float, n_cores: int = NCORES, stage: int = 8):
    """Emit + compile the SPMD program (same program on all cores).

    stage (debug bisect): 1=DMA+norms, 2=+sims matmul/exp, 3=+exp accum,
    4=+full phase B, 5=+text grind, 6=+RS2, 7=+vision grind+RS1, 8=full.
    """
    scale_b = 1.0 / eff_temp            # batch sims logits scale
    scale_q = queue_weight / eff_temp   # queue logits scale

    nc = bacc.Bacc(
        "TRN2", target_bir_lowering=False, debug=False, num_devices=n_cores
    )

    # ---- kernel I/O (per core) ----
    vfT_d = nc.dram_tensor("vfT", [D, B], F32R, kind="ExternalInput")
    tfT_d = nc.dram_tensor("tfT", [D, B], F32R, kind="ExternalInput")
    vfrkT_d = nc.dram_tensor("vf_rkT", [D, 128], F32R, kind="ExternalInput")
    tfrkT_d = nc.dram_tensor("tf_rkT", [D, 128], F32R, kind="ExternalInput")
    mid_d = nc.dram_tensor("mid", [128, B], F32, kind="ExternalInput")
    midrk_d = nc.dram_tensor("mid_rk", [128, 1], F32, kind="ExternalInput")
    tq_d = nc.dram_tensor("tq", [D, QS], F32R, kind="ExternalInput")
    vq_d = nc.dram_tensor("vq", [D, QS], F32R, kind="ExternalInput")
    out_d = nc.dram_tensor("partials", [128, 3], F32, kind="ExternalOutput")

    # ---- collective buffers (internal DRAM) ----
    # cc2: qsum_v partials, laid out [row_tile, lane] so ReduceScatter hands
    # core k the summed block for its own row shard.
    cc2_in = nc.dram_tensor("cc2_in", [RT, 128], F32)
    cc2_out = nc.dram_tensor("cc2_out", [1, 128], F32)
    # cc1: [row_tile, 2, lane] = (qsum_t, batch colsum) partials.
    cc1_in = nc.dram_tensor("cc1_in", [RT, 2, 128], F32)
    cc1_out = nc.dram_tensor("cc1_out", [2, 128], F32)

    rg = [list(range(n_cores))]

    with tile.TileContext(nc) as tc:
        with tc.tile_pool(name="sb", bufs=1) as sb:
            # persistent SBUF tiles
            vfT = sb.tile([D, B], F32R, tag="vfT")
            tfT = sb.tile([D, B], F32R, tag="tfT")
            vfrkT = sb.tile([D, 128], F32R, tag="vfrkT")
            tfrkT = sb.tile([D, 128], F32R, tag="tfrkT")
            midb = sb.tile([128, B], F32, tag="midb")
            midrk = sb.tile([128, 1], F32, tag="midrk")
            tq_sb = sb.tile([D, QS], F32R, tag="tq")
            vq_sb = sb.tile([D, QS], F32R, tag="vq")
            mask = sb.tile([128, B], F32, tag="mask")
            sqbuf = sb.tile([128, B], F32, tag="sqbuf")
            lnbuf = sb.tile([1, B], F32, tag="lnbuf")
            rnbuf = sb.tile([1, B], F32, tag="rnbuf")
            ones = sb.tile([128, 1], F32, tag="ones")
            nones = sb.tile([128, 1], F32, tag="nones")
            ones1 = sb.tile([1, 128], F32, tag="ones1")
            E_r = sb.tile([128, B], F32, tag="E_r")
            ET_c = sb.tile([128, B], F32, tag="ET_c")
            Em = sb.tile([128, B], F32, tag="Em")
            rsumE = sb.tile([128, 1], F32, tag="rsumE")
            possum = sb.tile([128, 1], F32, tag="possum")
            rnm = sb.tile([128, 1], F32, tag="rnm")
            cs_sb = sb.tile([1, B], F32, tag="cs_sb")
            np_rows = sb.tile([128, 1], F32, tag="np_rows")
            qacc_v = sb.tile([128, RT * NG], F32, tag="qacc_v")
            qacc_t = sb.tile([128, RT * NG], F32, tag="qacc_t")
            qsum_v = sb.tile([128, RT], F32, tag="qsum_v")
            qsum_t = sb.tile([128, RT], F32, tag="qsum_t")
            etrash = sb.tile([128, GW], F32, tag="etrash")
            trashA = sb.tile([128, B], F32, tag="trashA")
            trashB = sb.tile([128, B], F32, tag="trashB")
            qvt = sb.tile([128, 1], F32, tag="qvt")
            qtt = sb.tile([128, 1], F32, tag="qtt")
            cst = sb.tile([128, 1], F32, tag="cst")
            negv = sb.tile([128, 1], F32, tag="negv")
            negt = sb.tile([128, 1], F32, tag="negt")
            lsum_v = sb.tile([128, 1], F32, tag="lsum_v")
            lsum_t = sb.tile([128, 1], F32, tag="lsum_t")
            ssum_v = sb.tile([128, 1], F32, tag="ssum_v")
            ssum_t = sb.tile([128, 1], F32, tag="ssum_t")
            lv = sb.tile([128, 1], F32, tag="lv")
            lt = sb.tile([128, 1], F32, tag="lt")

            # ---------- input DMAs ----------
            nc.sync.dma_start(out=vfT[:, :], in_=vfT_d.ap()[:, :])
            nc.sync.dma_start(out=tfT[:, :], in_=tfT_d.ap()[:, :])
            nc.sync.dma_start(out=vfrkT[:, :], in_=vfrkT_d.ap()[:, :])
            nc.sync.dma_start(out=tfrkT[:, :], in_=tfrkT_d.ap()[:, :])
            nc.sync.dma_start(out=midb[:, :], in_=mid_d.ap()[:, :])
            nc.sync.dma_start(out=midrk[:, :], in_=midrk_d.ap()[:, :])
            # queue shards, chunked so compute can start early
            for c in range(NG):
                cs_ = slice(c * GW, (c + 1) * GW)
                nc.sync.dma_start(out=tq_sb[:, cs_], in_=tq_d.ap()[:, cs_])
            for c in range(NG):
                cs_ = slice(c * GW, (c + 1) * GW)
                nc.sync.dma_start(out=vq_sb[:, cs_], in_=vq_d.ap()[:, cs_])

            nc.vector.memset(ones[:, :], 1.0)
            nc.vector.memset(nones[:, :], -1.0)
            nc.vector.memset(ones1[:, :], 1.0)

            # ---------- phase A: l2-normalize features (in place) ----------
            def norm_chain(xT, n, psA):
                nc.vector.tensor_mul(sqbuf[:, :n], xT[:, :], xT[:, :])
                n2 = psA.tile([1, B], F32, tag="n2")
                for j in range(0, n, 512):
                    nc.tensor.matmul(
                        n2[:, j : j + 512],
                        ones[:, :],
                        sqbuf[:, j : j + 512],
                        start=True,
                        stop=True,
                    )
                # rnorm = exp(-0.5 * ln(norm2))  (avoids sqrt table load)
                nc.scalar.activation(lnbuf[:, :n], n2[:, :n], AF.Ln)
                nc.scalar.activation(rnbuf[:, :n], lnbuf[:, :n], AF.Exp, scale=-0.5)
                # broadcast rnorm across partitions via PE: ones1^T @ rnorm_row
                rb = psA.tile([128, B], F32, tag="rb")
                for j in range(0, n, 512):
                    nc.tensor.matmul(
                        rb[:, j : j + 512],
                        ones1[0:1, :],
                        rnbuf[0:1, j : j + 512],
                        start=True,
                        stop=True,
                    )
                # write the normalized features as float32r so the verifier
                # accepts them as fp32r-matmul inputs
                nc.vector.tensor_mul(_f32r(xT[:, :]), xT[:, :], rb[:, :n])

            with tc.tile_pool(name="psA", bufs=2, space="PSUM") as psA:
                norm_chain(vfT, B, psA)   # vision first: text-queue grind needs it
                norm_chain(tfT, B, psA)
                norm_chain(vfrkT, 128, psA)
                norm_chain(tfrkT, 128, psA)

            # match mask for this core's row/col shard: mask[p, j] =
            # (mid[rk_p] == mid[j])
            nc.vector.tensor_scalar(
                mask[:, :], midb[:, :], midrk[:, 0:1], None, ALU.is_equal
            )
            nc.vector.reduce_sum(np_rows[:, :], mask[:, :], axis=AX.X)

            # ---------- phase B: batch sims for own shard ----------
            if stage >= 2:
                with tc.tile_pool(name="psB", bufs=1, space="PSUM") as psB:
                    sims_r = psB.tile([128, B], F32, tag="sims_r")
                    simsT_c = psB.tile([128, B], F32, tag="simsT_c")
                    cs_ps = psB.tile([1, B], F32, tag="cs_ps")
                    for j in range(0, B, 512):
                        nc.tensor.matmul(
                            sims_r[:, j : j + 512],
                            _f32r(vfrkT[:, :]),
                            _f32r(tfT[:, j : j + 512]),
                            start=True,
                            stop=True,
                        )
                    nc.scalar.activation(
                        E_r[:, :],
                        sims_r[:, :],
                        AF.Exp,
                        scale=scale_b,
                        accum_out=rsumE[:, :] if stage >= 3 else None,
                    )
                    for j in range(0, B, 512):
                        nc.tensor.matmul(
                            simsT_c[:, j : j + 512],
                            _f32r(tfrkT[:, :]),
                            _f32r(vfT[:, j : j + 512]),
                            start=True,
                            stop=True,
                        )
                    nc.scalar.activation(
                        ET_c[:, :], simsT_c[:, :], AF.Exp, scale=scale_b
                    )

                    import os as _os

                    _sub = int(_os.environ.get("KSUB", "9"))
                    if stage >= 4 and _sub >= 1:
                        # Em = E_r * mask ; possum = rowsum(Em)
                        nc.vector.tensor_mul(Em[:, :], E_r[:, :], mask[:, :])
                        nc.vector.reduce_sum(possum[:, :], Em[:, :], axis=AX.X)
                        nc.vector.tensor_sub(rnm[:, :], rsumE[:, :], possum[:, :])
                    if stage >= 4 and _sub >= 2:
                        # batch colsums of non-matching exp(sims)
                        for j in range(0, B, 512):
                            nc.tensor.matmul(
                                cs_ps[:, j : j + 512],
                                ones[:, :],
                                E_r[:, j : j + 512],
                                start=True,
                                stop=False,
                            )
                            nc.tensor.matmul(
                                cs_ps[:, j : j + 512],
                                nones[:, :],
                                Em[:, j : j + 512],
                                start=False,
                                stop=True,
                            )
                        nc.vector.tensor_copy(cs_sb[:, :], cs_ps[:, :])
                    else:
                        nc.vector.tensor_copy(cs_sb[:, :], E_r[0:1, :])

            # ---------- queue grind ----------
            def grind(queue_sb, lhsT, qacc, qsum, pg):
                for r in range(RT):
                    lhs = _f32r(lhsT[:, r * 128 : (r + 1) * 128])
                    for c in range(NG):
                        ps = pg.tile([128, GW], F32, tag="gps")
                        for j in range(NMM):
                            col = c * GW + j * 512
                            nc.tensor.matmul(
                                ps[:, j * 512 : (j + 1) * 512],
                                lhs,
                                queue_sb[:, col : col + 512],
                                start=True,
                                stop=True,
                            )
                        idx = r * NG + c
                        nc.scalar.activation(
                            etrash[:, :],
                            ps[:, :],
                            AF.Exp,
                            scale=scale_q,
                            accum_out=qacc[:, idx : idx + 1],
                        )
                    nc.vector.reduce_sum(
                        qsum[:, r : r + 1],
                        qacc[:, r * NG : (r + 1) * NG],
                        axis=AX.X,
                    )

            if stage >= 5:
                # text queue -> qsum_v (feeds RS2)
                with tc.tile_pool(name="pgv", bufs=2, space="PSUM") as pg:
                    grind(tq_sb, vfT, qacc_v, qsum_v, pg)

            if stage >= 6:
                for r in range(RT):
                    nc.sync.dma_start(
                        out=cc2_in.ap()[r, :], in_=qsum_v[:, r : r + 1]
                    )
                nc.gpsimd.collective_compute(
                    "ReduceScatter",
                    ALU.add,
                    replica_groups=rg,
                    ins=[cc2_in.ap().opt()],
                    outs=[cc2_out.ap().opt()],
                )

            if stage >= 7:
                # vision queue -> qsum_t (feeds RS1)
                with tc.tile_pool(name="pgt", bufs=2, space="PSUM") as pg:
                    grind(vq_sb, tfT, qacc_t, qsum_t, pg)
                for r in range(RT):
                    nc.sync.dma_start(
                        out=cc1_in.ap()[r, 0, :], in_=qsum_t[:, r : r + 1]
                    )
                    nc.sync.dma_start(
                        out=cc1_in.ap()[r, 1, :],
                        in_=cs_sb[0:1, r * 128 : (r + 1) * 128],
                    )
                nc.gpsimd.collective_compute(
                    "ReduceScatter",
                    ALU.add,
                    replica_groups=rg,
                    ins=[cc1_in.ap().opt()],
                    outs=[cc1_out.ap().opt()],
                )

            if stage >= 8:
                # ---------- phase D: loss terms for own shard ----------
                with tc.tile_pool(name="psD", bufs=1, space="PSUM") as psD:
                    # v2t: rows shard.  neg_v = batch-nonmatch rowsum + queue
                    nc.sync.dma_start(out=qvt[:, :], in_=cc2_out.ap()[0, :])
                    nc.vector.tensor_add(negv[:, :], rnm[:, :], qvt[:, :])
                    nc.scalar.activation(
                        trashA[:, :], E_r[:, :], AF.Ln, bias=negv[:, 0:1]
                    )
                    nc.vector.tensor_mul(trashB[:, :], trashA[:, :], mask[:, :])
                    nc.vector.reduce_sum(lsum_v[:, :], trashB[:, :], axis=AX.X)
                    simsD = psD.tile([128, B], F32, tag="simsD")
                    for j in range(0, B, 512):
                        nc.tensor.matmul(
                            simsD[:, j : j + 512],
                            _f32r(vfrkT[:, :]),
                            _f32r(tfT[:, j : j + 512]),
                            start=True,
                            stop=True,
                        )
                    nc.vector.tensor_mul(trashB[:, :], simsD[:, :], mask[:, :])
                    nc.vector.reduce_sum(ssum_v[:, :], trashB[:, :], axis=AX.X)
                    # lv = lsum_v - scale_b * ssum_v
                    nc.vector.tensor_scalar(
                        ssum_v[:, :], ssum_v[:, :], scale_b, None, ALU.mult
                    )
                    nc.vector.tensor_sub(lv[:, :], lsum_v[:, :], ssum_v[:, :])

                    # t2v: cols shard.  neg_t = batch colsum + queue sum
                    nc.sync.dma_start(out=cst[:, :], in_=cc1_out.ap()[1, :])
                    nc.sync.dma_start(out=qtt[:, :], in_=cc1_out.ap()[0, :])
                    nc.vector.tensor_add(negt[:, :], cst[:, :], qtt[:, :])
                    nc.scalar.activation(
                        trashA[:, :], ET_c[:, :], AF.Ln, bias=negt[:, 0:1]
                    )
                    nc.vector.tensor_mul(trashB[:, :], trashA[:, :], mask[:, :])
                    nc.vector.reduce_sum(lsum_t[:, :], trashB[:, :], axis=AX.X)
                    simsTD = psD.tile([128, B], F32, tag="simsTD")
                    for j in range(0, B, 512):
                        nc.tensor.matmul(
                            simsTD[:, j : j + 512],
                            _f32r(tfrkT[:, :]),
                            _f32r(vfT[:, j : j + 512]),
                            start=True,
                            stop=True,
                        )
                    nc.vector.tensor_mul(trashB[:, :], simsTD[:, :], mask[:, :])
                    nc.vector.reduce_sum(ssum_t[:, :], trashB[:, :], axis=AX.X)
                    nc.vector.tensor_scalar(
                        ssum_t[:, :], ssum_t[:, :], scale_b, None, ALU.mult
                    )
                    nc.vector.tensor_sub(lt[:, :], lsum_t[:, :], ssum_t[:, :])

                # ---------- outputs ----------
                nc.sync.dma_start(out=out_d.ap()[:, 0:1], in_=lv[:, :])
                nc.sync.dma_start(out=out_d.ap()[:, 1:2], in_=lt[:, :])
                nc.sync.dma_start(out=out_d.ap()[:, 2:3], in_=np_rows[:, :])
            else:
                # debug stages: emit whatever is defined
                nc.sync.dma_start(out=out_d.ap()[:, 0:1], in_=np_rows[:, :])
                src1 = E_r if stage >= 2 else np_rows
                nc.sync.dma_start(out=out_d.ap()[:, 1:2], in_=src1[:, 0:1])
                src2 = qsum_v if stage >= 5 else np_rows
                nc.sync.dma_start(out=out_d.ap()[:, 2:3], in_=src2[:, 0:1])

    nc.compile()
    return nc


def schedule_scalars(fill_level: int):
    fill_ratio = min(int(fill_level), Q) / Q
    eff_temp = MAX_TEMP - (MAX_TEMP - INIT_TEMP) * fill_ratio
    if fill_ratio >= 0.95:
        eff_temp = INIT_TEMP
    queue_weight = min(1.0, fill_ratio * 1.5)
    if fill_ratio < 0.2:
        queue_weight = fill_ratio * 0.5
    return eff_temp, queue_weight


def make_in_maps(
    vision_features, text_features, match_ids, vision_queue, text_queue
):
    vf = np.asarray(vision_features, dtype=np.float32)
    tf_ = np.asarray(text_features, dtype=np.float32)
    vq = np.asarray(vision_queue, dtype=np.float32)
    tq = np.asarray(text_queue, dtype=np.float32)
    mid = np.asarray(match_ids).astype(np.float32)

    vfT = np.ascontiguousarray(vf.T)
    tfT = np.ascontiguousarray(tf_.T)
    mid_bcast = np.ascontiguousarray(np.broadcast_to(mid.reshape(1, B), (128, B)))

    in_maps = []
    for k in range(NCORES):
        rk = slice(k * 128, (k + 1) * 128)
        qs = slice(k * QS, (k + 1) * QS)
        in_maps.append(
            {
                "vfT": vfT,
                "tfT": tfT,
                "vf_rkT": np.ascontiguousarray(vf[rk].T),
                "tf_rkT": np.ascontiguousarray(tf_[rk].T),
                "mid": mid_bcast,
                "mid_rk": np.ascontiguousarray(mid[rk].reshape(128, 1)),
                "tq": np.ascontiguousarray(tq[:, qs]),
                "vq": np.ascontiguousarray(vq[:, qs]),
            }
        )
    return in_maps


def combine_partials(partials_list):
    """partials_list: NCORES arrays of [128, 3] -> scalar loss (fp32)."""
    P = np.stack([np.asarray(p, dtype=np.float64) for p in partials_list])
    s = P.sum(axis=(0, 1))  # [3] = (v2t, t2v, num_pos)
    loss = (s[0] / s[2] + s[1] / s[2]) / 2.0
    return np.float32(loss)


_NC_CACHE: dict = {}


def _get_compiled(eff_temp: float, queue_weight: float, stage: int = 8):
    key = (round(eff_temp, 9), round(queue_weight, 9), stage)
    if key not in _NC_CACHE:
        _NC_CACHE[key] = build(eff_temp, queue_weight, stage=stage)
    return _NC_CACHE[key]


def kernel(
    vision_features,
    text_features,
    match_ids,
    vision_queue,
    text_queue,
    fill_level,
    **_ignored,
):
    eff_temp, queue_weight = schedule_scalars(fill_level)
    nc = _get_compiled(eff_temp, queue_weight)
    in_maps = make_in_maps(
        vision_features, text_features, match_ids, vision_queue, text_queue
    )
    res = bass_utils.run_bass_kernel_spmd(
        nc, in_maps, core_ids=list(range(NCORES))
    )
    return combine_partials([r["partials"] for r in res.results])
